# revision 1
# baseline (speedup 1.0000x reference)
"""ComplEx rhs-scoring kernel for Trainium2 (8 NeuronCores).

scores = Re(<lhs * rel, conj(all_ents)>) = q @ ent_emb.T
where q = [q_re, q_im] (complex product of gathered lhs/rel embeddings).

Strategy (tensor-parallel over candidates):
  - host: gather + complex product -> q [B, K] (tiny, exact fp32),
    transpose to qT [K, B]; transpose ent_emb -> eT [K, N]; split eT
    into 8 column slabs [K, N/8] (one per core); replicate qT.
  - device (per core): scores_slab[b, n] = sum_k qT[k, b] * eT[k, n]
    via PE matmuls: lhsT = qT k-tile [128, 128], rhs = eT chunk
    [128, CW], accumulate K/128 = 8 matmuls into PSUM fp32.
  - host: concat slabs along axis 1 -> [B, N] fp32.
"""

import os
import numpy as np

import concourse.bacc as bacc
import concourse.mybir as mybir
import concourse.tile as tile
from concourse.bass_utils import run_bass_kernel_spmd

N_CORES = 8
B = 1024          # batch (queries)
K = 1024          # contraction dim (2 * rank)
N_ENT = 100000    # candidates
NS = N_ENT // N_CORES  # per-core slab width (12500)
P = 128           # partitions
KT = K // P       # k tiles (8)
BT = B // P       # b tiles (8)
CW = 500          # rhs chunk width (one PSUM bank, 25 chunks per slab)

_DT = {
    "bf16": mybir.dt.bfloat16,
    "f32r": mybir.dt.float32r,
    "f32": mybir.dt.float32,
}


def build_kernel(dt_name, ns=NS, cw=CW, b=B):
    dt_in = _DT[dt_name]
    f32 = mybir.dt.float32
    nc = bacc.Bacc("TRN2", target_bir_lowering=False, debug=False)

    qT = nc.dram_tensor("qT", [K, b], dt_in, kind="ExternalInput")
    eT = nc.dram_tensor("eT", [K, ns], dt_in, kind="ExternalInput")
    out = nc.dram_tensor("out", [b, ns], f32, kind="ExternalOutput")

    bt = b // P
    n_chunks = ns // cw
    assert n_chunks * cw == ns

    with tile.TileContext(nc) as tc:
        with (
            tc.tile_pool(name="qpool", bufs=1) as qpool,
            tc.tile_pool(name="epool", bufs=3) as epool,
            tc.tile_pool(name="pspool", bufs=8, space="PSUM") as pspool,
            tc.tile_pool(name="opool", bufs=4) as opool,
        ):
            q_tiles = []
            for k in range(KT):
                qt = qpool.tile([P, b], dt_in, tag=f"q{k}")
                nc.sync.dma_start(qt[:], qT[k * P:(k + 1) * P, :])
                q_tiles.append(qt)

            for c in range(n_chunks):
                et = epool.tile([P, KT * cw], dt_in)
                for k in range(KT):
                    nc.sync.dma_start(
                        et[:, k * cw:(k + 1) * cw],
                        eT[k * P:(k + 1) * P, c * cw:(c + 1) * cw],
                    )
                for bi in range(bt):
                    ps = pspool.tile([P, cw], f32)
                    for k in range(KT):
                        nc.tensor.matmul(
                            ps[:],
                            q_tiles[k][:, bi * P:(bi + 1) * P],
                            et[:, k * cw:(k + 1) * cw],
                            start=(k == 0),
                            stop=(k == KT - 1),
                        )
                    ot = opool.tile([P, cw], f32)
                    if bi % 2 == 0:
                        nc.vector.tensor_copy(ot[:], ps[:])
                    else:
                        nc.scalar.copy(ot[:], ps[:])
                    nc.sync.dma_start(
                        out[bi * P:(bi + 1) * P, c * cw:(c + 1) * cw], ot[:]
                    )
    nc.compile()
    return nc


def _prep_inputs(x, ent_emb, rel_emb, dt_name):
    x = np.asarray(x)
    ent_emb = np.asarray(ent_emb, dtype=np.float32)
    rel_emb = np.asarray(rel_emb, dtype=np.float32)
    r = ent_emb.shape[1] // 2
    lhs = ent_emb[x[:, 0]]
    rel = rel_emb[x[:, 1]]
    lre, lim = lhs[:, :r], lhs[:, r:]
    rre, rim = rel[:, :r], rel[:, r:]
    q = np.empty((x.shape[0], 2 * r), np.float32)
    q[:, :r] = lre * rre - lim * rim
    q[:, r:] = lre * rim + lim * rre

    if dt_name == "bf16":
        import ml_dtypes
        np_dt = ml_dtypes.bfloat16
    else:
        np_dt = np.float32

    qT = np.ascontiguousarray(q.T).astype(np_dt)           # [K, B]
    eT = np.ascontiguousarray(ent_emb.T).astype(np_dt)     # [K, N]
    in_maps = [
        {"qT": qT, "eT": np.ascontiguousarray(eT[:, i * NS:(i + 1) * NS])}
        for i in range(N_CORES)
    ]
    return in_maps


def run(x, ent_emb, rel_emb, dt_name=None, trace=False, **spmd_kwargs):
    dt_name = dt_name or os.environ.get("KERNEL_DT", "f32r")
    nc = build_kernel(dt_name)
    in_maps = _prep_inputs(x, ent_emb, rel_emb, dt_name)
    res = run_bass_kernel_spmd(
        nc, in_maps, list(range(N_CORES)), trace=trace, **spmd_kwargs
    )
    outs = [res.results[i]["out"] for i in range(N_CORES)]
    return np.concatenate(outs, axis=1), res


def kernel(x, ent_emb, rel_emb):
    out, _ = run(x, ent_emb, rel_emb)
    return out
